# revision 2
# baseline (speedup 1.0000x reference)
"""Trainium2 Bass kernel for nn_Confidence_Score (gnn_message_passing), v6.

Math: with S_g = sum of x over nodes of graph g and n_g = node count,
every node of graph g has identical activations:
    h1_g = relu(S_g @ W1 + b1)
    h2_g = relu((n_g * h1_g) @ W2 + b2)
    c_g  = h2_g @ Wc + bc ;  out_node = sp/(1+sp), sp = softplus(c_g)

v5 design (bf16 data, f32 accumulate):
 - DMA: the two HW queues (sync/scalar) run transfers serially at
   ~125GB/s each -> they carry ONLY x (2 groups each); all consts ride
   the gpsimd software DGE (slow but concurrent). iota/identity are
   generated on-device to keep the const tensor small.
 - pass 1: per 128-node chunk ONE matmul, lhsT=x_chunk (stationary),
   rhs=mask[128,NR] -> run sums [128, NR] at free-axis psum offsets.
 - run->graph: per 75-run slice: CAST + PE transpose + matmul with
   R[75,G] (counts n_g folded into R) accumulated into T^T = (n*S)^T,
   column-split into t_psA (graphs < GSPL, complete after slice
   n_rt-2) and t_psB (rest).
 - MLP in transposed layout; n*relu(z)=relu(n*z) folds the layer-2
   count scaling into pass 1; b1*n enters as a K=1 rank-1 matmul; b2 and
   bc ride activation biases. softplus = ln(exp(c+bc)+1) via one
   preloaded ACT table (Relu/Exp/Ln).
 - pass 2: chunks span <=3 graphs -> out[c,p] = og_lo[c] +
   d1[c]*(p>=b1[c]) + d2[c]*(p>=b2[c]); og_lo/mid/hi from 3 Sel-gather
   matmuls; is_ge step masks precomputed in the DMA window.
 - early/late pipeline: graphs < GSPL finish MLP+softplus and chunks
   < CSPL expand + DMA out while the last x group is still in flight;
   only the remainder runs after the final slice.

Sharding: graph-aligned contiguous node ranges, balanced by node count,
one range per core (8 cores); weights replicated; no collectives.
"""

import os
import sys

for _p in ("/root/.axon_site", "/root/.axon_site/_ro/trn_rl_repo",
           "/root/.axon_site/_ro/pypackages", "/opt/trn_rl_repo"):
    if os.path.isdir(_p) and _p not in sys.path:
        sys.path.append(_p)

import numpy as np

N_CORES = 8
D = 128
H = 256
G_TOTAL = 512
G_PAD = 72        # max local graphs per core (actual ~66)
CHUNK = 128       # nodes per aggregation matmul
XB = 20           # chunks per x DMA group
NR = 3            # mask/run columns per chunk
CPT = 25          # chunks per reduce slice (NR*CPT=75 runs <= 128)
ESL = 2           # slices whose graphs feed the early chain
GSPL = 32         # graph cols complete after slice ESL-1 (psum base ok)
CSPL = 44         # chunks touching only graphs < GSPL

# selw bf16 tile columns ([128, SELW]), ordered by DMA deadline:
W_W1 = 0                  # W1 [128,256]
W_WC = 256                # wc as 2 cols
W_SB = 258                # row-0 vectors: b1a|b1b|n (256+G_PAD cols)
W_W2A = 586               # W2 rows 0-127 [128,256]
W_W2B = 842               # W2 rows 128-255 [128,256]
W_R = 1098                # R slices [75, G_PAD] x n_rt (counts folded in)

_CACHE = {}


def _layout(n_chunks):
    n_rt = -(-n_chunks // CPT)
    SEL = W_R + n_rt * G_PAD            # Sel low block [32, n_chunks]x3
    SEL2 = SEL + 3 * n_chunks           # Sel high block [40, CL]x3
    SELW = SEL2 + 3 * (n_chunks - CSPL)
    return n_rt, SEL, SEL2, SELW


def _build(nodes_pad):
    """Build + compile the single-core Bass program (uniform across cores)."""
    from contextlib import ExitStack

    import concourse.bacc as bacc
    import concourse.mybir as mybir
    import concourse.tile as tile

    f32 = mybir.dt.float32
    bf16 = mybir.dt.bfloat16
    i16 = mybir.dt.int16
    i32 = mybir.dt.int32
    AF = mybir.ActivationFunctionType
    OP = mybir.AluOpType

    n_chunks = nodes_pad // CHUNK
    assert n_chunks % XB == 0
    n_groups = n_chunks // XB
    n_rt, SEL, SEL2, SELW = _layout(n_chunks)
    SB = W_SB
    RSL = NR * CPT
    GB = G_PAD - GSPL
    CL = n_chunks - CSPL

    nc = bacc.Bacc("TRN2", target_bir_lowering=False, debug=False)

    xb_d = nc.dram_tensor("xb", [nodes_pad, D], bf16, kind="ExternalInput").ap()
    mkn_d = nc.dram_tensor("mkn", [128, NR * n_chunks], bf16,
                           kind="ExternalInput").ap()
    selw_d = nc.dram_tensor("selw", [128, SELW], bf16,
                            kind="ExternalInput").ap()
    auxf_d = nc.dram_tensor("auxf", [128, 8], f32, kind="ExternalInput").ap()
    out_d = nc.dram_tensor("out", [n_chunks, CHUNK], bf16,
                           kind="ExternalOutput").ap()

    # host pre-shuffles xb so each (group, partition) segment is contiguous
    xb_groups = xb_d.rearrange("(g p j) d -> g p (j d)", p=CHUNK, j=XB)

    with tile.TileContext(nc) as tc, ExitStack() as ctx:
        const = ctx.enter_context(tc.tile_pool(name="const", bufs=1))
        ps_t = ctx.enter_context(tc.tile_pool(name="ps_t", bufs=1, space="PSUM"))
        mlp = ctx.enter_context(tc.tile_pool(name="mlp", bufs=1))
        ps_m = ctx.enter_context(tc.tile_pool(name="ps_m", bufs=1, space="PSUM"))

        # ACT table holding Exp+Ln (relu runs on vector); loaded on the
        # scalar queue after the x DMA issues so the table fetch doesn't
        # stall the x transfers
        from concourse.hw_specs import get_activation_tables
        need = {AF.Exp, AF.Ln}
        tid = next(i for i, (_, fns) in
                   enumerate(get_activation_tables(nc.m.arch).items())
                   if need <= fns)

        def load_act_table():
            nc.scalar.add_instruction(mybir.InstLoadActFuncSet(
                name=nc.get_next_instruction_name(), act_func_set_id=tid,
                ins=[], outs=[]))

        selw = const.tile([128, SELW], bf16)
        auxf = const.tile([128, 8], f32)
        mkn = const.tile([128, NR * n_chunks], bf16)
        # on-device iotas first on the gpsimd queue (before its DMA work)
        ioti = const.tile([128, 128], i16)
        nc.gpsimd.iota(ioti[:], pattern=[[1, 128]], base=0,
                       channel_multiplier=0)
        iotc = const.tile([128, 1], i32)
        nc.gpsimd.iota(iotc[:], pattern=[[0, 1]], base=0,
                       channel_multiplier=1)
        # consts by deadline: aux, masks, R, W1+biases, W2, Sel
        nc.gpsimd.dma_start(auxf[:], auxf_d[:])
        nc.gpsimd.dma_start(mkn[:], mkn_d[:])
        nc.gpsimd.dma_start(selw[:, W_R:SEL], selw_d[:, W_R:SEL])
        nc.gpsimd.dma_start(selw[:, 0:W_W2A], selw_d[:, 0:W_W2A])
        nc.gpsimd.dma_start(selw[:, W_W2A:W_R], selw_d[:, W_W2A:W_R])
        nc.gpsimd.dma_start(selw[:, SEL:SELW], selw_d[:, SEL:SELW])

        w1_s = selw[:, W_W1:W_W1 + H]
        w2a = selw[:, W_W2A:W_W2A + H]
        w2b = selw[:, W_W2B:W_W2B + H]
        wc2 = selw[:, W_WC:W_WC + 2]
        b1row = [selw[0:1, SB:SB + 128],
                 selw[0:1, SB + 128:SB + 256]]      # [1, 128] x2
        n_row = selw[0:1, SB + 256:SB + 256 + G_PAD]
        assert SB == W_SB
        bc_col = auxf[:, 0:1]                       # bc in every row
        b2act = [auxf[:, 3:4], auxf[:, 4:5]]        # [128,1] f32 ACT bias

        iotf = const.tile([128, 1], f32)
        nc.vector.tensor_copy(iotf[:], iotc[:])
        ident = const.tile([128, 128], bf16)
        nc.vector.tensor_scalar(ident[:], ioti[:], iotf[:], None,
                                op0=OP.is_equal)
        # step masks vs iota, split early/late (all tiles partition-base 0;
        # host duplicates the late boundary cols at rows 0:CL)
        s1f = [const.tile([CSPL, 128], bf16, tag="s1e", name="s1e"),
               const.tile([CL, 128], bf16, tag="s1l", name="s1l")]
        s2f = [const.tile([CSPL, 128], bf16, tag="s2e", name="s2e"),
               const.tile([CL, 128], bf16, tag="s2l", name="s2l")]

        def make_steps():
            for i, (w, c1, c2) in enumerate(((CSPL, 1, 2), (CL, 5, 6))):
                nc.vector.tensor_scalar(s1f[i][:], ioti[0:w, :],
                                        auxf[0:w, c1:c1 + 1], None,
                                        op0=OP.is_ge)
                nc.vector.tensor_scalar(s2f[i][:], ioti[0:w, :],
                                        auxf[0:w, c2:c2 + 1], None,
                                        op0=OP.is_ge)

        t_psA = ps_t.tile([128, GSPL], f32)
        t_psB = ps_t.tile([128, GB], f32)
        ogA = mlp.tile([GSPL, 1], bf16)
        ogB = mlp.tile([GB, 1], bf16)
        CW = max(CSPL, CL)
        og2_ps = ps_m.tile([CW, NR], f32)
        outb = mlp.tile([CW, 128], bf16)
        ogc = mlp.tile([CW, NR], f32)
        dd = mlp.tile([CW, 2], f32)
        o1f = mlp.tile([CW, 128], bf16)
        t2f = mlp.tile([CW, 128], bf16)

        def mlp_chain(gofs, w, tps, og_t):
            """h1/h2/c/softplus for graph columns [gofs, gofs+w) -> og_t.
            Every partition access stays at base 0."""
            lo, hi = gofs, gofs + w
            t_sb = mlp.tile([128, G_PAD], bf16, tag="t_sb", name="t_sb")
            nc.vector.tensor_copy(t_sb[:, lo:hi], tps)
            h1sb = []
            for k in range(2):
                hp = ps_m.tile([128, G_PAD], f32, tag=f"hp{k}",
                               name=f"hp{k}")
                nc.tensor.matmul(hp[:, 0:w],
                                 lhsT=w1_s[:, k * 128:(k + 1) * 128],
                                 rhs=t_sb[:, lo:hi], start=True, stop=False)
                nc.tensor.matmul(hp[:, 0:w], lhsT=b1row[k],
                                 rhs=n_row[:, lo:hi], start=False, stop=True)
                hs = mlp.tile([128, G_PAD], bf16, tag=f"h1s{k}",
                              name=f"h1s{k}")
                nc.vector.tensor_scalar_max(hs[:, 0:w], hp[:, 0:w], 0.0)
                h1sb.append(hs)
            h2sb = []
            for k in range(2):
                hp = ps_m.tile([128, G_PAD], f32, tag=f"hp{k}",
                               name=f"hp{k}b")
                nc.tensor.matmul(hp[:, 0:w],
                                 lhsT=w2a[:, k * 128:(k + 1) * 128],
                                 rhs=h1sb[0][:, 0:w], start=True, stop=False)
                nc.tensor.matmul(hp[:, 0:w],
                                 lhsT=w2b[:, k * 128:(k + 1) * 128],
                                 rhs=h1sb[1][:, 0:w], start=False, stop=True)
                hs = mlp.tile([128, G_PAD], bf16, tag=f"h2s{k}",
                              name=f"h2s{k}")
                nc.vector.tensor_scalar(hs[:, 0:w], hp[:, 0:w], b2act[k],
                                        0.0, op0=OP.add, op1=OP.max)
                h2sb.append(hs)
            c_ps = ps_m.tile([G_PAD, 1], f32, tag="cc", name="cc")
            nc.tensor.matmul(c_ps[0:w, :], lhsT=h2sb[0][:, 0:w],
                             rhs=wc2[:, 0:1], start=True, stop=False)
            nc.tensor.matmul(c_ps[0:w, :], lhsT=h2sb[1][:, 0:w],
                             rhs=wc2[:, 1:2], start=False, stop=True)
            # og = 1 - 1/(1 + ln(exp(c+bc)+1)); exp->inf saturates to 1
            ex = mlp.tile([G_PAD, 1], f32, tag="ex", name="ex")
            nc.scalar.activation(ex[0:w, :], c_ps[0:w, :], AF.Exp,
                                 bias=bc_col[0:w, :])
            sp = mlp.tile([G_PAD, 1], f32, tag="sp", name="sp")
            nc.scalar.activation(sp[0:w, :], ex[0:w, :], AF.Ln, bias=1.0)
            t1 = mlp.tile([G_PAD, 1], f32, tag="t1", name="t1")
            nc.vector.tensor_scalar_add(t1[0:w, :], sp[0:w, :], 1.0)
            rcp = mlp.tile([G_PAD, 1], f32, tag="rc", name="rc")
            nc.vector.reciprocal(rcp[0:w, :], t1[0:w, :])
            nc.vector.tensor_scalar(og_t[0:w, :], rcp[0:w, :], -1.0, 1.0,
                                    op0=OP.mult, op1=OP.add)

        def expand(i, clo, w, late, ring):
            """Sel-gather og2 for chunks [clo,clo+w), then step-expand
            and DMA out. All tiles at partition base 0."""
            for r in range(NR):
                nc.tensor.matmul(
                    og2_ps[0:w, r:r + 1],
                    lhsT=selw[0:GSPL, SEL + r * n_chunks + clo:
                              SEL + r * n_chunks + clo + w],
                    rhs=ogA[:], start=True, stop=not late)
                if late:
                    nc.tensor.matmul(
                        og2_ps[0:w, r:r + 1],
                        lhsT=selw[0:GB, SEL2 + r * CL:SEL2 + r * CL + w],
                        rhs=ogB[:], start=False, stop=True)
            nc.vector.tensor_copy(ogc[0:w, :], og2_ps[0:w, :])
            nc.vector.tensor_scalar(o1f[0:w, :], s1f[i][:],
                                    ogc[0:w, 1:2], ogc[0:w, 0:1],
                                    op0=OP.mult, op1=OP.add)
            nc.vector.tensor_scalar(t2f[0:w, :], s2f[i][:],
                                    ogc[0:w, 2:3], None, op0=OP.mult)
            nc.vector.tensor_tensor(outb[0:w, :], o1f[0:w, :],
                                    t2f[0:w, :], op=OP.add)
            ring.dma_start(out_d[clo:clo + w, :], outb[0:w, :])

        # ---- pass 1: per-chunk run sums [128, 3] at free-axis offsets ----
        n_done = [0]
        early = [False]
        with (
            tc.tile_pool(name="ps_r", bufs=1, space="PSUM") as ps_r,
            tc.tile_pool(name="ps_tp", bufs=1, space="PSUM") as ps_tp,
            tc.tile_pool(name="xp", bufs=n_groups) as xpool,
        ):
            sr_ps = ps_r.tile([128, NR * n_chunks], f32)

            def reduce_slice(q):
                off = RSL * q
                kk = min(RSL, NR * n_chunks - off)
                srb = const.tile([128, RSL], bf16, tag=f"srb{q}")
                nc.vector.tensor_copy(srb[:, 0:kk], sr_ps[:, off:off + kk])
                tp = ps_tp.tile([128, 128], bf16, tag="tp", name="tp")
                nc.tensor.transpose(tp[0:kk, 0:128], srb[:, 0:kk], ident[:])
                trb = const.tile([128, 128], bf16, tag=f"trb{q}")
                nc.vector.tensor_copy(trb[0:kk, :], tp[0:kk, 0:128])
                if q < ESL:
                    nc.tensor.matmul(
                        t_psA[:], lhsT=trb[0:kk, :],
                        rhs=selw[0:kk, W_R + q * G_PAD:
                                 W_R + q * G_PAD + GSPL],
                        start=(q == 0), stop=(q == ESL - 1))
                nc.tensor.matmul(
                    t_psB[:], lhsT=trb[0:kk, :],
                    rhs=selw[0:kk, W_R + q * G_PAD + GSPL:
                             W_R + (q + 1) * G_PAD],
                    start=(q == 0), stop=(q == n_rt - 1))
                n_done[0] += 1

            for g in range(n_groups):
                xt = xpool.tile([CHUNK, XB * D], bf16)
                eng = nc.sync if g % 2 == 0 else nc.scalar
                eng.dma_start(xt[:], xb_groups[g])
                if g == n_groups - 1:
                    load_act_table()
                for j in range(XB):
                    c = g * XB + j
                    nc.tensor.matmul(
                        sr_ps[:, NR * c:NR * (c + 1)],
                        lhsT=xt[:, j * D:(j + 1) * D],
                        rhs=mkn[:, NR * c:NR * (c + 1)],
                        start=True, stop=True,
                    )
                # emit completed slice reduces after this group's chunk
                # matmuls (slice 0 deliberately waits for group 1 so its
                # transpose never blocks group 1's matmuls on the queue)
                if g >= 1:
                    while n_done[0] < n_rt and \
                            (n_done[0] + 1) * CPT <= (g + 1) * XB:
                        reduce_slice(n_done[0])
                if n_done[0] >= ESL and not early[0]:
                    # early chain: graphs < GSPL complete after slices
                    # 0..ESL-1; run their MLP + softplus and expand the
                    # first CSPL chunks while later x groups are in flight
                    early[0] = True
                    make_steps()
                    mlp_chain(0, GSPL, t_psA, ogA)
                    expand(0, 0, CSPL, False, nc.sync)
            while n_done[0] < n_rt:
                reduce_slice(n_done[0])

        # ---- late chain: remaining graph columns + chunks ----
        mlp_chain(GSPL, GB, t_psB, ogB)
        expand(1, CSPL, CL, True, nc.scalar)

    nc.compile()
    return nc


def _shard(batch):
    """Graph-aligned split of nodes across cores, balanced by node count."""
    n = batch.shape[0]
    counts = np.bincount(batch, minlength=G_TOTAL).astype(np.int64)
    bounds = np.concatenate([[0], np.cumsum(counts)])
    gsplit = [0]
    for k in range(1, N_CORES):
        t = k * n // N_CORES
        g = int(np.searchsorted(bounds, t))
        if g > 0 and abs(int(bounds[g - 1]) - t) < abs(int(bounds[g]) - t):
            g -= 1
        g = min(max(g, gsplit[-1]), G_TOTAL)
        gsplit.append(g)
    gsplit.append(G_TOTAL)
    return counts, bounds, gsplit


def kernel(**inputs):
    import ml_dtypes
    from concourse.bass_utils import run_bass_kernel_spmd

    bf16 = ml_dtypes.bfloat16
    x = np.ascontiguousarray(np.asarray(inputs["x"], dtype=np.float32))
    batch = np.asarray(inputs["batch"]).astype(np.int64)
    W1 = np.asarray(inputs["W1"], dtype=np.float32)
    b1 = np.asarray(inputs["b1"], dtype=np.float32)
    W2 = np.asarray(inputs["W2"], dtype=np.float32)
    b2 = np.asarray(inputs["b2"], dtype=np.float32)
    Wc = np.asarray(inputs["Wc"], dtype=np.float32).reshape(H, 1)
    bc = np.asarray(inputs["bc"], dtype=np.float32).reshape(1)

    n = batch.shape[0]
    counts, bounds, gsplit = _shard(batch)
    node_cnt = [int(bounds[gsplit[k + 1]] - bounds[gsplit[k]])
                for k in range(N_CORES)]
    nodes_pad = int(-(-max(node_cnt) // (CHUNK * XB)) * (CHUNK * XB))
    assert max(gsplit[k + 1] - gsplit[k] for k in range(N_CORES)) <= G_PAD

    n_chunks = nodes_pad // CHUNK
    n_rt, SEL, SEL2, SELW = _layout(n_chunks)
    SB = W_SB

    key = nodes_pad
    if key not in _CACHE:
        _CACHE[key] = _build(nodes_pad)
    nc = _CACHE[key]

    selw0 = np.zeros((128, SELW), dtype=bf16)
    selw0[:, W_W1:W_W1 + H] = W1.astype(bf16)
    selw0[:, W_W2A:W_W2A + H] = W2[0:128].astype(bf16)
    selw0[:, W_W2B:W_W2B + H] = W2[128:256].astype(bf16)
    selw0[:, W_WC] = Wc[0:128, 0].astype(bf16)
    selw0[:, W_WC + 1] = Wc[128:256, 0].astype(bf16)
    selw0[0, SB:SB + H] = b1.astype(bf16)

    n_groups = nodes_pad // (CHUNK * XB)
    in_maps = []
    for k in range(N_CORES):
        gs, ge = gsplit[k], gsplit[k + 1]
        ns, ne = int(bounds[gs]), int(bounds[ge])
        cnt = ne - ns
        bt = np.full(nodes_pad, G_PAD - 1, dtype=np.int64)
        bt[:cnt] = batch[ns:ne] - gs
        xbp = np.zeros((nodes_pad, D), dtype=bf16)
        xbp[:cnt] = x[ns:ne].astype(bf16)
        # shuffle to (group, partition, chunk-in-group, row) DMA order
        xbp = np.ascontiguousarray(
            xbp.reshape(n_groups, XB, CHUNK, D).transpose(0, 2, 1, 3)
        ).reshape(nodes_pad, D)

        mkn = np.zeros((128, NR * n_chunks), dtype=bf16)
        selw = selw0.copy()
        selw[0, SB + 256:SB + 256 + (ge - gs)] = counts[gs:ge].astype(bf16)
        Rm = np.zeros((n_rt, NR * CPT, G_PAD), dtype=np.float32)
        sel3 = np.zeros((3, G_PAD, n_chunks), dtype=np.float32)
        bnd = np.full((n_chunks, 2), 128.0, dtype=np.float32)
        btc = bt.reshape(n_chunks, CHUNK)
        for c in range(n_chunks):
            lo = c * CHUNK
            if lo >= cnt:
                sel3[0, G_PAD - 1, c] = 1.0  # pure-pad chunk
                continue
            valid = min(CHUNK, cnt - lo)
            row = btc[c][:valid]
            glo, ghi = int(row[0]), int(row[valid - 1])
            nrr = ghi - glo + 1
            assert nrr <= NR, f"chunk {c} spans {nrr} graphs > NR"
            # early/late split safety (program structure relies on these)
            if c < CSPL:
                assert ghi < GSPL, f"chunk {c} touches graph {ghi} >= {GSPL}"
            if c >= ESL * CPT:
                assert glo >= GSPL, f"chunk {c} touches graph {glo} < {GSPL}"
            for r in range(nrr):
                mkn[:valid, NR * c + r] = (row == glo + r).astype(np.float32)
                Rm[c // CPT, NR * (c % CPT) + r, glo + r] = counts[gs + glo + r]
            jumps = np.flatnonzero(np.diff(row)) + 1
            assert len(jumps) <= 2, f"chunk {c} has {len(jumps)} boundaries"
            gseq = [glo]
            for bi, j in enumerate(jumps):
                bnd[c, bi] = float(j)
                gseq.append(int(row[j]))
            while len(gseq) < 3:
                gseq.append(gseq[-1])
            sel3[0, gseq[0], c] = 1.0
            sel3[1, gseq[1], c] += 1.0
            sel3[1, gseq[0], c] -= 1.0
            sel3[2, gseq[2], c] += 1.0
            sel3[2, gseq[1], c] -= 1.0
        selw[0:NR * CPT, W_R:W_R + n_rt * G_PAD] = np.ascontiguousarray(
            Rm.transpose(1, 0, 2).reshape(NR * CPT, n_rt * G_PAD)).astype(bf16)
        CL = n_chunks - CSPL
        assert sel3[:, GSPL:, :CSPL].sum() == 0
        selw[0:GSPL, SEL:SEL + 3 * n_chunks] = np.ascontiguousarray(
            sel3[:, 0:GSPL, :].transpose(1, 0, 2).reshape(
                GSPL, 3 * n_chunks)).astype(bf16)
        selw[0:G_PAD - GSPL, SEL2:SEL2 + 3 * CL] = np.ascontiguousarray(
            sel3[:, GSPL:, CSPL:].transpose(1, 0, 2).reshape(
                G_PAD - GSPL, 3 * CL)).astype(bf16)

        auxf = np.zeros((128, 8), dtype=np.float32)
        auxf[:, 0] = bc[0]
        auxf[0:CSPL, 1] = bnd[0:CSPL, 0]
        auxf[0:CSPL, 2] = bnd[0:CSPL, 1]
        auxf[0:n_chunks - CSPL, 5] = bnd[CSPL:, 0]
        auxf[0:n_chunks - CSPL, 6] = bnd[CSPL:, 1]
        auxf[:, 3] = b2[0:128]
        auxf[:, 4] = b2[128:256]

        in_maps.append({"xb": xbp, "mkn": mkn, "selw": selw, "auxf": auxf})

    res = run_bass_kernel_spmd(nc, in_maps, core_ids=list(range(N_CORES)))
    outs = []
    for k in range(N_CORES):
        o = res.results[k]["out"].reshape(-1)
        outs.append(o[: node_cnt[k]])
    return np.concatenate(outs).reshape(n, 1).astype(np.float32)
